# revision 12
# baseline (speedup 1.0000x reference)
"""Trainium2 Bass kernel for the attention module.

Math (per batch row b):
    q   = hidden @ W1.T                      [B, D]
    att = einsum('bsd,bd->bs', enc, q)       [B, S]
    masked = att + (-1e10 where s >= len)    (additive mask is exact here:
                                              |att| < ulp(1e10) so att-1e10
                                              rounds to -1e10, matching the
                                              reference's masked==0 -> -1e10)
    attn = softmax(masked, axis=s)
    ctx = einsum('bs,bsd->bd', attn, enc)
    out = tanh(concat([ctx, hidden], 1) @ W2.T)
    returns (out, attn.T)

Distribution: data-parallel over batch across 8 cores (4 rows each), W1/W2
replicated. Per core the kernel streams encoder_outs[b] (8 MB) into SBUF once,
computes scores with a fused DVE multiply+reduce against a partition-broadcast
q, does the masked softmax with GPSIMD partition reductions, then re-uses the
cached tiles for the context matvecs on the tensor engine (block-diagonal
attention columns so all 4 batch rows accumulate in one PSUM tile).
"""

import sys

for _p in ("/opt/trn_rl_repo",):
    if _p not in sys.path:
        sys.path.insert(0, _p)

import numpy as np

B, S, D, H = 32, 2048, 1024, 1024
NCORES = 8
BL = B // NCORES          # 4 batch rows per core
NS = S // 128             # 16 score columns (128 s-values each)
NCHUNK = 8                # enc DMA chunks per batch row (256 s-rows = 1 MB)
A = (S // NCHUNK) // 128  # 2 sub-tiles per chunk

_built = {}


def _build(niter: int = 1):
    """Build + compile the 8-core SPMD bass program. Cached per niter."""
    if niter in _built:
        return _built[niter]

    from concourse import bacc, mybir
    import concourse.tile as tile
    from concourse import bass_isa
    from concourse.masks import make_identity

    f32 = mybir.dt.float32
    AF = mybir.ActivationFunctionType
    ALU = mybir.AluOpType
    RED = bass_isa.ReduceOp

    nc = bacc.Bacc("TRN2", target_bir_lowering=False, debug=False,
                   num_devices=NCORES)

    enc = nc.dram_tensor("enc", [BL, S, D], f32, kind="ExternalInput").ap()
    hidT = nc.dram_tensor("hidT", [H, BL], f32, kind="ExternalInput").ap()
    maskT = nc.dram_tensor("maskT", [128, BL, NS], f32, kind="ExternalInput").ap()
    w1t = nc.dram_tensor("w1t", [H, D], f32, kind="ExternalInput").ap()
    w2t = nc.dram_tensor("w2t", [D + H, D], f32, kind="ExternalInput").ap()
    sels_d = nc.dram_tensor("sels", [BL, BL, 128], f32, kind="ExternalInput").ap()
    ident_d = nc.dram_tensor("ident", [128, 128], f32, kind="ExternalInput").ap()
    out_d = nc.dram_tensor("out", [BL, D], f32, kind="ExternalOutput").ap()
    attn_d = nc.dram_tensor("attn", [BL, S], f32, kind="ExternalOutput").ap()

    with tile.TileContext(nc) as tc:
        for _ in range(niter):
            _emit_one(nc, tc, mybir, f32, AF, ALU, RED, sels_d, ident_d,
                      enc, hidT, maskT, w1t, w2t, out_d, attn_d)

    nc.compile()
    _built[niter] = nc
    return nc


def _emit_one(nc, tc, mybir, f32, AF, ALU, RED, sels_d, ident_d,
              enc, hidT, maskT, w1t, w2t, out_d, attn_d):
    from contextlib import ExitStack

    with ExitStack() as ctx:
        const = ctx.enter_context(tc.tile_pool(name="const", bufs=1))
        ident = const.tile([128, 128], f32)
        nc.sync.dma_start(ident[:], ident_d[:])
        sels_sb = const.tile([BL, BL, 128], f32)
        nc.sync.dma_start(sels_sb[:], sels_d[:])
        hid_sb = const.tile([128, H // 128, BL], f32)
        nc.sync.dma_start(hid_sb[:], hidT.rearrange("(j p) b -> p j b", p=128))
        mask_sb = const.tile([128, BL, NS], f32)
        nc.sync.dma_start(mask_sb[:], maskT[:])

        # ---- q = hidden @ W1.T, then broadcast each row across partitions
        qb = const.tile([128, BL, D], f32)      # q[b] replicated on all parts
        q_sb = const.tile([BL, D], f32)
        with tc.tile_pool(name="w1p", bufs=3) as w1p, \
             tc.tile_pool(name="psq", bufs=2, space="PSUM") as psq:
            q_ps = [psq.tile([BL, 512], f32, tag=f"q{i}", name=f"q_ps{i}") for i in range(2)]
            for hj in range(H // 128):
                w1c = w1p.tile([128, D], f32)
                nc.sync.dma_start(w1c[:], w1t[hj * 128:(hj + 1) * 128, :])
                for nh in range(2):
                    nc.tensor.matmul(q_ps[nh][:], lhsT=hid_sb[:, hj, :],
                                     rhs=w1c[:, nh * 512:(nh + 1) * 512],
                                     start=(hj == 0), stop=(hj == H // 128 - 1))
            for nh in range(2):
                nc.scalar.copy(q_sb[:, nh * 512:(nh + 1) * 512], q_ps[nh][:])
            for b in range(BL):
                for nh in range(2):
                    qb_ps = psq.tile([128, 512], f32, tag=f"qb{nh}",
                                     name=f"qb_ps_{b}_{nh}")
                    nc.tensor.matmul(qb_ps[:], lhsT=sels_sb[:, b, :],
                                     rhs=q_sb[:, nh * 512:(nh + 1) * 512],
                                     start=True, stop=True)
                    nc.scalar.copy(qb[:, b, nh * 512:(nh + 1) * 512], qb_ps[:])

        # ---- pools for the main streaming loop
        encp = ctx.enter_context(tc.tile_pool(name="encp", bufs=10))
        w2p = ctx.enter_context(tc.tile_pool(name="w2p", bufs=16))
        small = ctx.enter_context(tc.tile_pool(name="small", bufs=2))
        bdp = ctx.enter_context(tc.tile_pool(name="bdp", bufs=2))
        pctx = ctx.enter_context(tc.tile_pool(name="pctx", bufs=1, space="PSUM"))
        ptp = ctx.enter_context(tc.tile_pool(name="ptp", bufs=2, space="PSUM"))
        outs = ctx.enter_context(tc.tile_pool(name="outs", bufs=1))

        prodp = ctx.enter_context(tc.tile_pool(name="prodp", bufs=3))
        junk = outs.tile([128, D], f32)         # ACT main output (discarded)
        attn_nat = outs.tile([16, BL, 128], f32)
        ctx_ps = [pctx.tile([BL, 512], f32, tag=f"ctx{i}", name=f"ctx_ps{i}") for i in range(2)]
        w2_tiles = []

        for b in range(BL):
            # stream this row's enc tiles; fused mul+reduce -> scores
            scoresT = small.tile([128, NS], f32, tag="scoresT")
            etiles = []
            for ci in range(NCHUNK):
                et = encp.tile([128, A, D], f32, tag="enc")
                src = enc[b, ci * A * 128:(ci + 1) * A * 128, :]
                nc.sync.dma_start(et[:], src.rearrange("(a p) d -> p a d", a=A))
                etiles.append(et)
                for a in range(A):
                    c = ci * A + a
                    prod = prodp.tile([128, D], f32, tag="prod")
                    nc.vector.tensor_mul(prod[:], et[:, a, :], qb[:, b, :])
                    nc.scalar.activation(junk[:], prod[:], AF.Copy,
                                         accum_out=scoresT[:, c:c + 1])

            # W2 prefetch goes out right after the first row's enc stream
            if b == 1:
                for j in range((D + H) // 128):
                    w2c = w2p.tile([128, D], f32, tag="w2")
                    nc.sync.dma_start(w2c[:], w2t[j * 128:(j + 1) * 128, :])
                    w2_tiles.append(w2c)

            # masked softmax over all S for this row
            msk = small.tile([128, NS], f32, tag="msk")
            nc.vector.tensor_add(msk[:], scoresT[:], mask_sb[:, b, :])
            red = small.tile([128, 1], f32, tag="red")
            nc.vector.tensor_reduce(red[:], msk[:], axis=mybir.AxisListType.X,
                                    op=ALU.max)
            nc.gpsimd.partition_all_reduce(red[:], red[:], 128, RED.max)
            negmax = small.tile([128, 1], f32, tag="negmax")
            nc.scalar.mul(negmax[:], red[:], -1.0)
            expT = small.tile([128, NS], f32, tag="expT")
            nc.scalar.activation(expT[:], msk[:], AF.Exp, bias=negmax[:])
            zsum = small.tile([128, 1], f32, tag="zsum")
            nc.vector.tensor_reduce(zsum[:], expT[:], axis=mybir.AxisListType.X,
                                    op=ALU.add)
            nc.gpsimd.partition_all_reduce(zsum[:], zsum[:], 128, RED.add)
            zinv = small.tile([128, 1], f32, tag="zinv")
            nc.vector.reciprocal(zinv[:], zsum[:])

            # normalized attention into a block-diagonal [128, NS, BL] (col b)
            bd = bdp.tile([128, NS, BL], f32, tag="bd")
            nc.vector.memset(bd[:], 0.0)
            nc.vector.tensor_scalar_mul(bd[:, :, b], expT[:], zinv[:])

            # attn row to natural [c, p] layout for a packed DMA at the end
            tp = ptp.tile([16, 128], f32, tag="tp")
            nc.tensor.transpose(tp[:], bd[:, :, b], ident[:])
            nc.vector.tensor_copy(attn_nat[:, b, :], tp[:])

            # context matvecs: all rows accumulate into one PSUM pair
            for ci in range(NCHUNK):
                for a in range(A):
                    c = ci * A + a
                    for dh in range(2):
                        nc.tensor.matmul(
                            ctx_ps[dh][:],
                            lhsT=bd[:, c, :],
                            rhs=etiles[ci][:, a, dh * 512:(dh + 1) * 512],
                            start=(b == 0 and c == 0),
                            stop=(b == BL - 1 and c == NS - 1))

        # ---- epilogue: ctx -> ctxT, out = tanh(concat @ W2.T)
        ctx_sb = outs.tile([BL, D], f32)
        for dh in range(2):
            nc.scalar.copy(ctx_sb[:, dh * 512:(dh + 1) * 512], ctx_ps[dh][:])
        ctxT = outs.tile([128, D // 128, BL], f32)
        for j in range(D // 128):
            tpj = ptp.tile([128, BL], f32, tag="tpj")
            nc.tensor.transpose(tpj[:], ctx_sb[:, j * 128:(j + 1) * 128],
                                ident[0:BL, 0:BL])
            nc.vector.tensor_copy(ctxT[:, j, :], tpj[:])

        with tc.tile_pool(name="pso", bufs=1, space="PSUM") as pso:
            o_ps = [pso.tile([BL, 512], f32, tag=f"o{i}", name=f"o_ps{i}") for i in range(2)]
            NJ = (D + H) // 128
            for j in range(NJ):
                lhsT = ctxT[:, j, :] if j < D // 128 else hid_sb[:, j - D // 128, :]
                for nh in range(2):
                    nc.tensor.matmul(o_ps[nh][:], lhsT=lhsT,
                                     rhs=w2_tiles[j][:, nh * 512:(nh + 1) * 512],
                                     start=(j == 0), stop=(j == NJ - 1))
            out_sb = outs.tile([BL, D], f32)
            for nh in range(2):
                nc.scalar.activation(out_sb[:, nh * 512:(nh + 1) * 512],
                                     o_ps[nh][:], AF.Tanh)

        nc.sync.dma_start(out_d[:], out_sb[:])
        nc.sync.dma_start(attn_d.rearrange("b (c p) -> c b p", c=16), attn_nat[:])


def _prepare_in_maps(hidden, encoder_outs, src_lens, W1, W2):
    hidden = np.asarray(hidden, np.float32)
    encoder_outs = np.ascontiguousarray(np.asarray(encoder_outs, np.float32))
    src_lens = np.asarray(src_lens)
    W1 = np.asarray(W1, np.float32)
    W2 = np.asarray(W2, np.float32)

    w1t = np.ascontiguousarray(W1.T)                       # [H, D]
    w2t = np.ascontiguousarray(W2.T)                       # [D+H, D]
    # additive mask, laid out as maskT[p, b_local, c] with s = c*128 + p
    mask = np.where(np.arange(S)[None, :] < src_lens[:, None], 0.0,
                    -1e10).astype(np.float32)              # [B, S]
    sels = np.zeros((BL, BL, 128), np.float32)
    for b in range(BL):
        sels[b, b, :] = 1.0
    # stored as [k, b, m] so sels_sb[:, b, :] selects matrix b
    sels = np.ascontiguousarray(sels.transpose(1, 0, 2))
    ident = np.eye(128, dtype=np.float32)

    in_maps = []
    for i in range(NCORES):
        sl = slice(i * BL, (i + 1) * BL)
        m = mask[sl].reshape(BL, NS, 128).transpose(2, 0, 1)  # [128, BL, NS]
        in_maps.append({
            "enc": np.ascontiguousarray(encoder_outs[sl]),
            "hidT": np.ascontiguousarray(hidden[sl].T),
            "maskT": np.ascontiguousarray(m),
            "w1t": w1t,
            "w2t": w2t,
            "sels": sels,
            "ident": ident,
        })
    return in_maps


def kernel(hidden, encoder_outs, src_lens, W1, W2):
    from concourse.bass_utils import run_bass_kernel_spmd

    nc = _build(1)
    in_maps = _prepare_in_maps(hidden, encoder_outs, src_lens, W1, W2)
    res = run_bass_kernel_spmd(nc, in_maps, list(range(NCORES))).results
    out = np.concatenate([res[i]["out"] for i in range(NCORES)], axis=0)
    attn = np.concatenate([res[i]["attn"] for i in range(NCORES)], axis=0)
    return out, np.ascontiguousarray(attn.T)


# revision 20
# speedup vs baseline: 6.3340x; 6.3340x over previous
"""Trainium2 Bass kernel for the attention module.

Math (per batch row b):
    q   = hidden @ W1.T                      [B, D]
    att = einsum('bsd,bd->bs', enc, q)       [B, S]
    masked = att + (-1e10 where s >= len)    (additive mask is exact here:
                                              |att| < ulp(1e10) so att-1e10
                                              rounds to -1e10, matching the
                                              reference's masked==0 -> -1e10)
    attn = softmax(masked, axis=s)
    ctx = einsum('bs,bsd->bd', attn, enc)
    out = tanh(concat([ctx, hidden], 1) @ W2.T)
    returns (out, attn.T)

Distribution: data-parallel over batch across 8 cores (4 rows each), W1/W2
replicated. Per core the kernel streams encoder_outs[b] (8 MB) into SBUF once,
computes scores with a fused DVE multiply+reduce against a partition-broadcast
q, does the masked softmax with GPSIMD partition reductions, then re-uses the
cached tiles for the context matvecs on the tensor engine (block-diagonal
attention columns so all 4 batch rows accumulate in one PSUM tile).
"""

import sys

for _p in ("/opt/trn_rl_repo",):
    if _p not in sys.path:
        sys.path.insert(0, _p)

import numpy as np

B, S, D, H = 32, 2048, 1024, 1024
NCORES = 8
BL = B // NCORES          # 4 batch rows per core
NS = S // 128             # 16 score columns (128 s-values each)
NCHUNK = 8                # enc DMA chunks per batch row (256 s-rows = 1 MB)
A = (S // NCHUNK) // 128  # 2 sub-tiles per chunk

_built = {}


def _build(niter: int = 1, mode: str = "full"):
    """Build + compile the 8-core SPMD bass program. Cached per (niter, mode)."""
    key = (niter, mode)
    if key in _built:
        return _built[key]

    from concourse import bacc, mybir
    import concourse.tile as tile
    from concourse import bass_isa
    from concourse.masks import make_identity

    f32 = mybir.dt.float32
    AF = mybir.ActivationFunctionType
    ALU = mybir.AluOpType
    RED = bass_isa.ReduceOp

    nc = bacc.Bacc("TRN2", target_bir_lowering=False, debug=False,
                   num_devices=NCORES)

    enc = nc.dram_tensor("enc", [BL, S, D], f32, kind="ExternalInput").ap()
    hidT = nc.dram_tensor("hidT", [H, BL], f32, kind="ExternalInput").ap()
    maskT = nc.dram_tensor("maskT", [128, BL, NS], f32, kind="ExternalInput").ap()
    w1t = nc.dram_tensor("w1t", [H, D], f32, kind="ExternalInput").ap()
    w2t = nc.dram_tensor("w2t", [D + H, D], f32, kind="ExternalInput").ap()
    sels_d = nc.dram_tensor("sels", [BL, BL, 128], f32, kind="ExternalInput").ap()
    ident_d = nc.dram_tensor("ident", [128, 128], f32, kind="ExternalInput").ap()
    out_d = nc.dram_tensor("out", [BL, D], f32, kind="ExternalOutput").ap()
    attn_d = nc.dram_tensor("attn", [BL, S], f32, kind="ExternalOutput").ap()

    with tile.TileContext(nc) as tc:
        for _ in range(niter):
            _emit_one(nc, tc, mybir, f32, AF, ALU, RED, sels_d, ident_d,
                      enc, hidT, maskT, w1t, w2t, out_d, attn_d, mode)

    nc.compile()
    _built[key] = nc
    return nc


def _emit_one(nc, tc, mybir, f32, AF, ALU, RED, sels_d, ident_d,
              enc, hidT, maskT, w1t, w2t, out_d, attn_d, mode="full"):
    from contextlib import ExitStack

    with ExitStack() as ctx:
        const = ctx.enter_context(tc.tile_pool(name="const", bufs=1))
        ident = const.tile([128, 128], f32)
        nc.sync.dma_start(ident[:], ident_d[:])
        sels_sb = const.tile([BL, BL, 128], f32)
        nc.sync.dma_start(sels_sb[:], sels_d[:])
        hid_sb = const.tile([128, H // 128, BL], f32)
        nc.sync.dma_start(hid_sb[:], hidT.rearrange("(j p) b -> p j b", p=128))
        mask_sb = const.tile([128, BL, NS], f32)
        nc.sync.dma_start(mask_sb[:], maskT[:])

        # ---- q = hidden @ W1.T, then broadcast each row across partitions.
        # qb holds q[b] twice along the free dim so one FD=2048 multiply
        # covers a whole [128, 2, 1024] enc chunk.
        qb = const.tile([128, BL, A, D], f32)
        q_sb = const.tile([BL, D], f32)
        with tc.tile_pool(name="w1p", bufs=3) as w1p, \
             tc.tile_pool(name="psq", bufs=2, space="PSUM") as psq:
            q_ps = [psq.tile([BL, 512], f32, tag=f"q{i}", name=f"q_ps{i}") for i in range(2)]
            for hj in range(H // 128):
                w1c = w1p.tile([128, D], f32)
                nc.sync.dma_start(w1c[:], w1t[hj * 128:(hj + 1) * 128, :])
                for nh in range(2):
                    nc.tensor.matmul(q_ps[nh][:], lhsT=hid_sb[:, hj, :],
                                     rhs=w1c[:, nh * 512:(nh + 1) * 512],
                                     start=(hj == 0), stop=(hj == H // 128 - 1))
            for nh in range(2):
                nc.scalar.copy(q_sb[:, nh * 512:(nh + 1) * 512], q_ps[nh][:])
            for b in range(BL):
                for nh in range(2):
                    qb_ps = psq.tile([128, 512], f32, tag=f"qb{nh}",
                                     name=f"qb_ps_{b}_{nh}")
                    nc.tensor.matmul(qb_ps[:], lhsT=sels_sb[:, b, :],
                                     rhs=q_sb[:, nh * 512:(nh + 1) * 512],
                                     start=True, stop=True)
                    for a in range(A):
                        nc.scalar.copy(
                            qb[:, b, a, nh * 512:(nh + 1) * 512], qb_ps[:])

        # ---- pools for the main streaming loop
        encp = ctx.enter_context(tc.tile_pool(name="encp", bufs=10))
        w2p = ctx.enter_context(tc.tile_pool(name="w2p", bufs=12))
        small = ctx.enter_context(tc.tile_pool(name="small", bufs=2))
        bdp = ctx.enter_context(tc.tile_pool(name="bdp", bufs=2))
        pctx = ctx.enter_context(tc.tile_pool(name="pctx", bufs=1, space="PSUM"))
        ptp = ctx.enter_context(tc.tile_pool(name="ptp", bufs=2, space="PSUM"))
        outs = ctx.enter_context(tc.tile_pool(name="outs", bufs=1))

        prodp = ctx.enter_context(tc.tile_pool(name="prodp", bufs=3))
        junk = outs.tile([128, D], f32)         # ACT main output (discarded)
        attn_nat = outs.tile([16, BL, 128], f32)
        ctx_ps = [pctx.tile([BL, 512], f32, tag=f"ctx{i}", name=f"ctx_ps{i}") for i in range(2)]
        w2_tiles = []

        for b in range(BL):
            # stream this row's enc tiles; fused mul+reduce -> scores
            scoresT = small.tile([128, NS], f32, tag="scoresT")
            etiles = []
            for ci in range(NCHUNK):
                et = encp.tile([128, A, D], f32, tag="enc")
                src = enc[b, ci * A * 128:(ci + 1) * A * 128, :]
                nc.sync.dma_start(et[:], src.rearrange("(a p) d -> p a d", a=A))
                etiles.append(et)
                if mode == "dma":
                    for a in range(A):
                        nc.vector.tensor_copy(scoresT[:, ci * A + a:ci * A + a + 1],
                                              et[:, a, 0:1])
                    continue
                if mode == "redonly":
                    for a in range(A):
                        c = ci * A + a
                        nc.scalar.activation(junk[:], et[:, a, :], AF.Copy,
                                             accum_out=scoresT[:, c:c + 1])
                    continue
                # whole-chunk multiply, alternating engines; ACT reduces
                prod = prodp.tile([128, A, D], f32, tag="prod")
                if mode == "gpmul":
                    eng = nc.gpsimd
                elif mode == "mulonly":
                    eng = nc.vector
                else:
                    eng = nc.vector if ci % 2 == 0 else nc.gpsimd
                eng.tensor_mul(prod[:], et[:], qb[:, b, :, :])
                if mode in ("mulonly", "gpmul"):
                    for a in range(A):
                        nc.vector.tensor_copy(
                            scoresT[:, ci * A + a:ci * A + a + 1],
                            prod[:, a, 0:1])
                    continue
                for a in range(A):
                    c = ci * A + a
                    nc.scalar.activation(junk[:], prod[:, a, :], AF.Copy,
                                         accum_out=scoresT[:, c:c + 1])

            # W2 prefetch goes out right after the first row's enc stream
            if b == 1:
                for j in range((D + H) // 128):
                    w2c = w2p.tile([128, D], f32, tag="w2")
                    nc.sync.dma_start(w2c[:], w2t[j * 128:(j + 1) * 128, :])
                    w2_tiles.append(w2c)

            if mode in ("dma", "scores", "redonly", "mulonly", "gpmul"):
                continue

            # masked softmax over all S for this row
            msk = small.tile([128, NS], f32, tag="msk")
            nc.vector.tensor_add(msk[:], scoresT[:], mask_sb[:, b, :])
            red = small.tile([128, 1], f32, tag="red")
            nc.vector.tensor_reduce(red[:], msk[:], axis=mybir.AxisListType.X,
                                    op=ALU.max)
            if mode != "nogp":
                nc.gpsimd.partition_all_reduce(red[:], red[:], 128, RED.max)
            negmax = small.tile([128, 1], f32, tag="negmax")
            nc.scalar.mul(negmax[:], red[:], -1.0)
            expT = small.tile([128, NS], f32, tag="expT")
            nc.scalar.activation(expT[:], msk[:], AF.Exp, bias=negmax[:])
            zsum = small.tile([128, 1], f32, tag="zsum")
            nc.vector.tensor_reduce(zsum[:], expT[:], axis=mybir.AxisListType.X,
                                    op=ALU.add)
            if mode != "nogp":
                nc.gpsimd.partition_all_reduce(zsum[:], zsum[:], 128, RED.add)
            zinv = small.tile([128, 1], f32, tag="zinv")
            nc.vector.reciprocal(zinv[:], zsum[:])

            # normalized attention into a block-diagonal [128, NS, BL] (col b)
            bd = bdp.tile([128, NS, BL], f32, tag="bd")
            nc.vector.memset(bd[:], 0.0)
            nc.vector.tensor_scalar_mul(bd[:, :, b], expT[:], zinv[:])

            # attn row to natural [c, p] layout for a packed DMA at the end
            tp = ptp.tile([16, 128], f32, tag="tp")
            nc.tensor.transpose(tp[:], bd[:, :, b], ident[:])
            nc.vector.tensor_copy(attn_nat[:, b, :], tp[:])

            # context matvecs: all rows accumulate into one PSUM pair
            if mode == "noctx":
                continue
            for ci in range(NCHUNK):
                for a in range(A):
                    c = ci * A + a
                    for dh in range(2):
                        nc.tensor.matmul(
                            ctx_ps[dh][:],
                            lhsT=bd[:, c, :],
                            rhs=etiles[ci][:, a, dh * 512:(dh + 1) * 512],
                            start=(b == 0 and c == 0),
                            stop=(b == BL - 1 and c == NS - 1))

        # ---- epilogue: ctx -> ctxT, out = tanh(concat @ W2.T)
        if mode in ("dma", "scores", "noctx", "redonly", "mulonly", "gpmul"):
            out_sb = outs.tile([BL, D], f32)
            nc.vector.memset(out_sb[:], 0.0)
            if mode != "noctx":
                nc.vector.memset(attn_nat[:], 0.0)
            nc.sync.dma_start(out_d[:], out_sb[:])
            nc.sync.dma_start(attn_d.rearrange("b (c p) -> c b p", c=16),
                              attn_nat[:])
            return
        ctx_sb = outs.tile([BL, D], f32)
        for dh in range(2):
            nc.scalar.copy(ctx_sb[:, dh * 512:(dh + 1) * 512], ctx_ps[dh][:])
        ctxT = outs.tile([128, D // 128, BL], f32)
        for j in range(D // 128):
            tpj = ptp.tile([128, BL], f32, tag="tpj")
            nc.tensor.transpose(tpj[:], ctx_sb[:, j * 128:(j + 1) * 128],
                                ident[0:BL, 0:BL])
            nc.vector.tensor_copy(ctxT[:, j, :], tpj[:])

        with tc.tile_pool(name="pso", bufs=1, space="PSUM") as pso:
            o_ps = [pso.tile([BL, 512], f32, tag=f"o{i}", name=f"o_ps{i}") for i in range(2)]
            NJ = (D + H) // 128
            for j in range(NJ):
                lhsT = ctxT[:, j, :] if j < D // 128 else hid_sb[:, j - D // 128, :]
                for nh in range(2):
                    nc.tensor.matmul(o_ps[nh][:], lhsT=lhsT,
                                     rhs=w2_tiles[j][:, nh * 512:(nh + 1) * 512],
                                     start=(j == 0), stop=(j == NJ - 1))
            out_sb = outs.tile([BL, D], f32)
            for nh in range(2):
                nc.scalar.activation(out_sb[:, nh * 512:(nh + 1) * 512],
                                     o_ps[nh][:], AF.Tanh)

        nc.sync.dma_start(out_d[:], out_sb[:])
        nc.sync.dma_start(attn_d.rearrange("b (c p) -> c b p", c=16), attn_nat[:])


def _prepare_in_maps(hidden, encoder_outs, src_lens, W1, W2):
    hidden = np.asarray(hidden, np.float32)
    encoder_outs = np.ascontiguousarray(np.asarray(encoder_outs, np.float32))
    src_lens = np.asarray(src_lens)
    W1 = np.asarray(W1, np.float32)
    W2 = np.asarray(W2, np.float32)

    w1t = np.ascontiguousarray(W1.T)                       # [H, D]
    w2t = np.ascontiguousarray(W2.T)                       # [D+H, D]
    # additive mask, laid out as maskT[p, b_local, c] with s = c*128 + p
    mask = np.where(np.arange(S)[None, :] < src_lens[:, None], 0.0,
                    -1e10).astype(np.float32)              # [B, S]
    sels = np.zeros((BL, BL, 128), np.float32)
    for b in range(BL):
        sels[b, b, :] = 1.0
    # stored as [k, b, m] so sels_sb[:, b, :] selects matrix b
    sels = np.ascontiguousarray(sels.transpose(1, 0, 2))
    ident = np.eye(128, dtype=np.float32)

    in_maps = []
    for i in range(NCORES):
        sl = slice(i * BL, (i + 1) * BL)
        m = mask[sl].reshape(BL, NS, 128).transpose(2, 0, 1)  # [128, BL, NS]
        in_maps.append({
            "enc": np.ascontiguousarray(encoder_outs[sl]),
            "hidT": np.ascontiguousarray(hidden[sl].T),
            "maskT": np.ascontiguousarray(m),
            "w1t": w1t,
            "w2t": w2t,
            "sels": sels,
            "ident": ident,
        })
    return in_maps


def kernel(hidden, encoder_outs, src_lens, W1, W2):
    from concourse.bass_utils import run_bass_kernel_spmd

    nc = _build(1)
    in_maps = _prepare_in_maps(hidden, encoder_outs, src_lens, W1, W2)
    res = run_bass_kernel_spmd(nc, in_maps, list(range(NCORES))).results
    out = np.concatenate([res[i]["out"] for i in range(NCORES)], axis=0)
    attn = np.concatenate([res[i]["attn"] for i in range(NCORES)], axis=0)
    return out, np.ascontiguousarray(attn.T)


# revision 24
# speedup vs baseline: 8.7555x; 1.3823x over previous
"""Trainium2 Bass kernel for the attention module.

Math (per batch row b):
    q   = hidden @ W1.T                      [B, D]
    att = einsum('bsd,bd->bs', enc, q)       [B, S]
    masked = att + (-1e10 where s >= len)    (additive mask is exact here:
                                              |att| < ulp(1e10) so att-1e10
                                              rounds to -1e10, matching the
                                              reference's masked==0 -> -1e10)
    attn = softmax(masked, axis=s)
    ctx = einsum('bs,bsd->bd', attn, enc)
    out = tanh(concat([ctx, hidden], 1) @ W2.T)
    returns (out, attn.T)

Distribution: data-parallel over batch across 8 cores (4 rows each), W1/W2
replicated. Per core the kernel streams encoder_outs[b] (8 MB) into SBUF once,
computes scores with a fused DVE multiply+reduce against a partition-broadcast
q, does the masked softmax with GPSIMD partition reductions, then re-uses the
cached tiles for the context matvecs on the tensor engine (block-diagonal
attention columns so all 4 batch rows accumulate in one PSUM tile).
"""

import sys

for _p in ("/opt/trn_rl_repo",):
    if _p not in sys.path:
        sys.path.insert(0, _p)

import numpy as np

B, S, D, H = 32, 2048, 1024, 1024
NCORES = 8
BL = B // NCORES          # 4 batch rows per core
NS = S // 128             # 16 score columns (128 s-values each)
NCHUNK = 8                # enc DMA chunks per batch row (256 s-rows = 1 MB)
A = (S // NCHUNK) // 128  # 2 sub-tiles per chunk

_built = {}


def _build(niter: int = 1, mode: str = "full"):
    """Build + compile the 8-core SPMD bass program. Cached per (niter, mode)."""
    key = (niter, mode)
    if key in _built:
        return _built[key]

    from concourse import bacc, mybir
    import concourse.tile as tile
    from concourse import bass_isa
    from concourse.masks import make_identity

    f32 = mybir.dt.float32
    AF = mybir.ActivationFunctionType
    ALU = mybir.AluOpType
    RED = bass_isa.ReduceOp

    nc = bacc.Bacc("TRN2", target_bir_lowering=False, debug=False,
                   num_devices=NCORES)

    enc = nc.dram_tensor("enc", [BL, S, D], f32, kind="ExternalInput").ap()
    hidT = nc.dram_tensor("hidT", [H, BL], f32, kind="ExternalInput").ap()
    maskT = nc.dram_tensor("maskT", [128, BL, NS], f32, kind="ExternalInput").ap()
    w1t = nc.dram_tensor("w1t", [H, D], f32, kind="ExternalInput").ap()
    w2t = nc.dram_tensor("w2t", [D + H, D], f32, kind="ExternalInput").ap()
    sels_d = nc.dram_tensor("sels", [BL, BL, 128], f32, kind="ExternalInput").ap()
    ident_d = nc.dram_tensor("ident", [128, 128], f32, kind="ExternalInput").ap()
    out_d = nc.dram_tensor("out", [BL, D], f32, kind="ExternalOutput").ap()
    attn_d = nc.dram_tensor("attn", [BL, S], f32, kind="ExternalOutput").ap()

    with tile.TileContext(nc) as tc:
        for _ in range(niter):
            _emit_one(nc, tc, mybir, f32, AF, ALU, RED, sels_d, ident_d,
                      enc, hidT, maskT, w1t, w2t, out_d, attn_d, mode)

    nc.compile()
    _built[key] = nc
    return nc


def _emit_one(nc, tc, mybir, f32, AF, ALU, RED, sels_d, ident_d,
              enc, hidT, maskT, w1t, w2t, out_d, attn_d, mode="full"):
    from contextlib import ExitStack

    with ExitStack() as ctx:
        const = ctx.enter_context(tc.tile_pool(name="const", bufs=1))
        ident = const.tile([128, 128], f32)
        nc.sync.dma_start(ident[:], ident_d[:])
        sels_sb = const.tile([BL, BL, 128], f32)
        nc.sync.dma_start(sels_sb[:], sels_d[:])
        hid_sb = const.tile([128, H // 128, BL], f32)
        nc.sync.dma_start(hid_sb[:], hidT.rearrange("(j p) b -> p j b", p=128))
        mask_sb = const.tile([128, BL, NS], f32)
        nc.sync.dma_start(mask_sb[:], maskT[:])

        # ---- q = hidden @ W1.T, then broadcast each row across partitions.
        # qb holds q[b] twice along the free dim so one FD=2048 multiply
        # covers a whole [128, 2, 1024] enc chunk.
        qb = const.tile([128, BL, A, D], f32)
        q_sb = const.tile([BL, D], f32)
        with tc.tile_pool(name="w1p", bufs=3) as w1p, \
             tc.tile_pool(name="psq", bufs=2, space="PSUM") as psq:
            q_ps = [psq.tile([BL, 512], f32, tag=f"q{i}", name=f"q_ps{i}") for i in range(2)]
            for hj in range(H // 128):
                w1c = w1p.tile([128, D], f32)
                nc.sync.dma_start(w1c[:], w1t[hj * 128:(hj + 1) * 128, :])
                for nh in range(2):
                    nc.tensor.matmul(q_ps[nh][:], lhsT=hid_sb[:, hj, :],
                                     rhs=w1c[:, nh * 512:(nh + 1) * 512],
                                     start=(hj == 0), stop=(hj == H // 128 - 1))
            for nh in range(2):
                nc.scalar.copy(q_sb[:, nh * 512:(nh + 1) * 512], q_ps[nh][:])
            for b in range(BL):
                for nh in range(2):
                    qb_ps = psq.tile([128, 512], f32, tag=f"qb{nh}",
                                     name=f"qb_ps_{b}_{nh}")
                    nc.tensor.matmul(qb_ps[:], lhsT=sels_sb[:, b, :],
                                     rhs=q_sb[:, nh * 512:(nh + 1) * 512],
                                     start=True, stop=True)
                    for a in range(A):
                        nc.scalar.copy(
                            qb[:, b, a, nh * 512:(nh + 1) * 512], qb_ps[:])

        # ---- pools for the main streaming loop
        encp = ctx.enter_context(tc.tile_pool(name="encp", bufs=10))
        w2p = ctx.enter_context(tc.tile_pool(name="w2p", bufs=12))
        small = ctx.enter_context(tc.tile_pool(name="small", bufs=2))
        bdp = ctx.enter_context(tc.tile_pool(name="bdp", bufs=2))
        pctx = ctx.enter_context(tc.tile_pool(name="pctx", bufs=1, space="PSUM"))
        ptp = ctx.enter_context(tc.tile_pool(name="ptp", bufs=2, space="PSUM"))
        outs = ctx.enter_context(tc.tile_pool(name="outs", bufs=1))

        prodp = ctx.enter_context(tc.tile_pool(name="prodp", bufs=3))
        ptiny = ctx.enter_context(tc.tile_pool(name="ptiny", bufs=2, space="PSUM"))
        junk = outs.tile([128, D], f32)         # ACT main output (discarded)
        ones_sb = outs.tile([128, 128], f32)    # all-ones (partition reductions)
        nc.vector.memset(ones_sb[:], 1.0)
        attn_nat = outs.tile([16, BL, 128], f32)
        ctx_ps = [pctx.tile([BL, 512], f32, tag=f"ctx{i}", name=f"ctx_ps{i}") for i in range(2)]
        w2_tiles = []

        for b in range(BL):
            # stream this row's enc tiles; fused mul+reduce -> scores
            scoresT = small.tile([128, NS], f32, tag="scoresT")
            etiles = []
            for ci in range(NCHUNK):
                et = encp.tile([128, A, D], f32, tag="enc")
                src = enc[b, ci * A * 128:(ci + 1) * A * 128, :]
                nc.sync.dma_start(et[:], src.rearrange("(a p) d -> p a d", a=A))
                etiles.append(et)
                if mode == "dma":
                    for a in range(A):
                        nc.vector.tensor_copy(scoresT[:, ci * A + a:ci * A + a + 1],
                                              et[:, a, 0:1])
                    continue
                if mode == "redonly":
                    for a in range(A):
                        c = ci * A + a
                        nc.scalar.activation(junk[:], et[:, a, :], AF.Copy,
                                             accum_out=scoresT[:, c:c + 1])
                    continue
                # whole-chunk multiply. GPSIMD handles FD=2048 chunks well;
                # DVE degrades above FD=1024, so it gets per-subtile ops.
                prod = prodp.tile([128, A, D], f32, tag="prod")
                if mode == "gpmul" or mode == "allgp":
                    use_dve = False
                elif mode == "mulonly":
                    use_dve = True
                else:
                    use_dve = ci in (0, 3, 6)
                if use_dve:
                    for a in range(A):
                        nc.vector.tensor_mul(prod[:, a, :], et[:, a, :],
                                             qb[:, b, a, :])
                else:
                    nc.gpsimd.tensor_mul(prod[:], et[:], qb[:, b, :, :])
                if mode in ("mulonly", "gpmul"):
                    for a in range(A):
                        nc.vector.tensor_copy(
                            scoresT[:, ci * A + a:ci * A + a + 1],
                            prod[:, a, 0:1])
                    continue
                for a in range(A):
                    c = ci * A + a
                    nc.scalar.activation(junk[:], prod[:, a, :], AF.Copy,
                                         accum_out=scoresT[:, c:c + 1])

            # W2 prefetch goes out right after the first row's enc stream
            if b == 1:
                for j in range((D + H) // 128):
                    w2c = w2p.tile([128, D], f32, tag="w2")
                    nc.sync.dma_start(w2c[:], w2t[j * 128:(j + 1) * 128, :])
                    w2_tiles.append(w2c)

            if mode in ("dma", "scores", "redonly", "mulonly", "gpmul", "allgp"):
                continue

            # masked softmax over all S for this row. Partition reductions go
            # through PE (transpose / ones-matmul); GPSIMD stays on muls.
            msk = small.tile([128, NS], f32, tag="msk")
            nc.vector.tensor_add(msk[:], scoresT[:], mask_sb[:, b, :])
            pm1 = ptiny.tile([16, 128], f32, tag="ptiny", name=f"pm1_{b}")
            nc.tensor.transpose(pm1[:], msk[:], ident[:])
            m16 = small.tile([16, 1], f32, tag="m16")
            nc.vector.tensor_reduce(m16[:], pm1[:], axis=mybir.AxisListType.X,
                                    op=ALU.max)
            pm2 = ptiny.tile([1, 16], f32, tag="ptiny", name=f"pm2_{b}")
            nc.tensor.transpose(pm2[:], m16[:], ident[0:16, 0:16])
            m1 = small.tile([1, 1], f32, tag="m1")
            nc.vector.tensor_reduce(m1[:], pm2[:], axis=mybir.AxisListType.X,
                                    op=ALU.max)
            pbc = ptiny.tile([128, 1], f32, tag="ptiny", name=f"pbc_{b}")
            nc.tensor.matmul(pbc[:], lhsT=ones_sb[0:1, :], rhs=m1[:],
                             start=True, stop=True)
            negmax = small.tile([128, 1], f32, tag="negmax")
            nc.scalar.mul(negmax[:], pbc[:], -1.0)
            expT = small.tile([128, NS], f32, tag="expT")
            nc.scalar.activation(expT[:], msk[:], AF.Exp, bias=negmax[:])
            pz1 = ptiny.tile([1, 16], f32, tag="ptiny", name=f"pz1_{b}")
            nc.tensor.matmul(pz1[:], lhsT=ones_sb[:, 0:1], rhs=expT[:],
                             start=True, stop=True)
            z1 = small.tile([1, 1], f32, tag="z1")
            nc.vector.tensor_reduce(z1[:], pz1[:], axis=mybir.AxisListType.X,
                                    op=ALU.add)
            zr = small.tile([1, 1], f32, tag="zr")
            nc.vector.reciprocal(zr[:], z1[:])
            pzb = ptiny.tile([128, 1], f32, tag="ptiny", name=f"pzb_{b}")
            nc.tensor.matmul(pzb[:], lhsT=ones_sb[0:1, :], rhs=zr[:],
                             start=True, stop=True)
            zinv = small.tile([128, 1], f32, tag="zinv")
            nc.scalar.copy(zinv[:], pzb[:])

            # normalized attention into a block-diagonal [128, NS, BL] (col b)
            bd = bdp.tile([128, NS, BL], f32, tag="bd")
            nc.vector.memset(bd[:], 0.0)
            nc.vector.tensor_scalar_mul(bd[:, :, b], expT[:], zinv[:])

            # attn row to natural [c, p] layout for a packed DMA at the end
            tp = ptp.tile([16, 128], f32, tag="tp")
            nc.tensor.transpose(tp[:], bd[:, :, b], ident[:])
            nc.vector.tensor_copy(attn_nat[:, b, :], tp[:])

            # context matvecs: all rows accumulate into one PSUM pair
            if mode == "noctx":
                continue
            for ci in range(NCHUNK):
                for a in range(A):
                    c = ci * A + a
                    for dh in range(2):
                        nc.tensor.matmul(
                            ctx_ps[dh][:],
                            lhsT=bd[:, c, :],
                            rhs=etiles[ci][:, a, dh * 512:(dh + 1) * 512],
                            start=(b == 0 and c == 0),
                            stop=(b == BL - 1 and c == NS - 1))

        # ---- epilogue: ctx -> ctxT, out = tanh(concat @ W2.T)
        if mode in ("dma", "scores", "noctx", "redonly", "mulonly", "gpmul", "allgp"):
            out_sb = outs.tile([BL, D], f32)
            nc.vector.memset(out_sb[:], 0.0)
            if mode != "noctx":
                nc.vector.memset(attn_nat[:], 0.0)
            nc.sync.dma_start(out_d[:], out_sb[:])
            nc.sync.dma_start(attn_d.rearrange("b (c p) -> c b p", c=16),
                              attn_nat[:])
            return
        ctx_sb = outs.tile([BL, D], f32)
        for dh in range(2):
            nc.scalar.copy(ctx_sb[:, dh * 512:(dh + 1) * 512], ctx_ps[dh][:])
        ctxT = outs.tile([128, D // 128, BL], f32)
        for j in range(D // 128):
            tpj = ptp.tile([128, BL], f32, tag="tp")
            nc.tensor.transpose(tpj[:], ctx_sb[:, j * 128:(j + 1) * 128],
                                ident[0:BL, 0:BL])
            nc.vector.tensor_copy(ctxT[:, j, :], tpj[:])

        with tc.tile_pool(name="pso", bufs=1, space="PSUM") as pso:
            o_ps = [pso.tile([BL, 512], f32, tag=f"o{i}", name=f"o_ps{i}") for i in range(2)]
            NJ = (D + H) // 128
            for j in range(NJ):
                lhsT = ctxT[:, j, :] if j < D // 128 else hid_sb[:, j - D // 128, :]
                for nh in range(2):
                    nc.tensor.matmul(o_ps[nh][:], lhsT=lhsT,
                                     rhs=w2_tiles[j][:, nh * 512:(nh + 1) * 512],
                                     start=(j == 0), stop=(j == NJ - 1))
            out_sb = outs.tile([BL, D], f32)
            for nh in range(2):
                nc.scalar.activation(out_sb[:, nh * 512:(nh + 1) * 512],
                                     o_ps[nh][:], AF.Tanh)

        nc.sync.dma_start(out_d[:], out_sb[:])
        nc.sync.dma_start(attn_d.rearrange("b (c p) -> c b p", c=16), attn_nat[:])


def _prepare_in_maps(hidden, encoder_outs, src_lens, W1, W2):
    hidden = np.asarray(hidden, np.float32)
    encoder_outs = np.ascontiguousarray(np.asarray(encoder_outs, np.float32))
    src_lens = np.asarray(src_lens)
    W1 = np.asarray(W1, np.float32)
    W2 = np.asarray(W2, np.float32)

    w1t = np.ascontiguousarray(W1.T)                       # [H, D]
    w2t = np.ascontiguousarray(W2.T)                       # [D+H, D]
    # additive mask, laid out as maskT[p, b_local, c] with s = c*128 + p
    mask = np.where(np.arange(S)[None, :] < src_lens[:, None], 0.0,
                    -1e10).astype(np.float32)              # [B, S]
    sels = np.zeros((BL, BL, 128), np.float32)
    for b in range(BL):
        sels[b, b, :] = 1.0
    # stored as [k, b, m] so sels_sb[:, b, :] selects matrix b
    sels = np.ascontiguousarray(sels.transpose(1, 0, 2))
    ident = np.eye(128, dtype=np.float32)

    in_maps = []
    for i in range(NCORES):
        sl = slice(i * BL, (i + 1) * BL)
        m = mask[sl].reshape(BL, NS, 128).transpose(2, 0, 1)  # [128, BL, NS]
        in_maps.append({
            "enc": np.ascontiguousarray(encoder_outs[sl]),
            "hidT": np.ascontiguousarray(hidden[sl].T),
            "maskT": np.ascontiguousarray(m),
            "w1t": w1t,
            "w2t": w2t,
            "sels": sels,
            "ident": ident,
        })
    return in_maps


def kernel(hidden, encoder_outs, src_lens, W1, W2):
    from concourse.bass_utils import run_bass_kernel_spmd

    nc = _build(1)
    in_maps = _prepare_in_maps(hidden, encoder_outs, src_lens, W1, W2)
    res = run_bass_kernel_spmd(nc, in_maps, list(range(NCORES))).results
    out = np.concatenate([res[i]["out"] for i in range(NCORES)], axis=0)
    attn = np.concatenate([res[i]["attn"] for i in range(NCORES)], axis=0)
    return out, np.ascontiguousarray(attn.T)
